# revision 46
# baseline (speedup 1.0000x reference)
"""AggrGATConv Trainium2 kernel: 8-core edge-parallel GAT with dst-sorted
window aggregation.

Design:
  inv-1 (device, node-sharded): h = feat @ W (plain fp32 PE matmul, near-
        full precision: 2 half-speed passes), el = h.Al, er = h.Ar fused as
        extra columns of the weight matrix; combined [h | el,er] table
        stored partition-major (128 DMA descriptors, host un-permutes).
  host: index-only edge prep (argsort by dst, degree-snake-balanced
        128-node windows, padding to full 128-edge tiles) + row gathers of
        device tables (pure data movement, no float arithmetic).
  inv-2 (device, edge-sharded): per window accumulate in PSUM
        [U | s] = sum_tiles B_t.T @ [expE*h | expE] (fp32 matmuls; exact
        because B is one-hot), then out = mean_heads(U / s) + mean(bias).
        One-hot B built 7 tiles on ScalarE (Abs/Relu trick) + 9 on Vector
        (is_equal) to balance engines; U/s normalization via a single
        Vector broadcast-multiply. Softmax uses exp without max-subtraction
        (logits are O(1) for this operator's scale).
"""
import math
import sys
import types
import contextlib
import ctypes

import numpy as np

import concourse.bacc as bacc
import concourse.tile as tile
import concourse.mybir as mybir
from concourse.bass import IndirectOffsetOnAxis  # noqa: F401
from concourse.bass_utils import run_bass_kernel_spmd

# ---------------- constants (hardcoded per problem spec) ----------------
N = 100000
E = 1600000
IN = 128
H, D = 4, 32
HD = H * D  # 128
NEG = 0.2
NCORES = 8
P = 128
WIN_PER_CORE = 98            # 98*128 = 12544 nodes per core
N_PAD = NCORES * WIN_PER_CORE * P  # 100352
NODES_PER_CORE = WIN_PER_CORE * P  # 12544

f32 = mybir.dt.float32
i32 = mybir.dt.int32


def _install_ntff_shim():
    """antenv.axon_hooks is absent in this image; provide the ctypes hook so
    trace=True works (used by test harness; harmless otherwise)."""
    if "antenv.axon_hooks" in sys.modules:
        return
    try:
        lib = ctypes.CDLL("/opt/axon/libaxon_pjrt.so")
        if not hasattr(lib, "axon_start_nrt_profile"):
            raise OSError("no symbol")
        lib.axon_start_nrt_profile.argtypes = [
            ctypes.POINTER(ctypes.c_int64), ctypes.c_size_t]
        lib.axon_start_nrt_profile.restype = ctypes.c_int64
        lib.axon_stop_nrt_profile.argtypes = [ctypes.c_char_p]
        lib.axon_stop_nrt_profile.restype = ctypes.c_int64

        @contextlib.contextmanager
        def _hook(output_dir, device_ids):
            import jax
            jax.devices()
            if device_ids:
                ids = (ctypes.c_int64 * len(device_ids))(*device_ids)
                rc = lib.axon_start_nrt_profile(ids, len(device_ids))
            else:
                rc = lib.axon_start_nrt_profile(None, 0)
            if rc != 0:
                raise RuntimeError(f"axon_start_nrt_profile rc={rc}")
            try:
                yield
            finally:
                n = lib.axon_stop_nrt_profile(str(output_dir).encode())
                print(f"profile: {n} file(s) -> {output_dir}", file=sys.stderr)

        hook = _hook
    except OSError:
        hook = None
    mod = types.ModuleType("antenv.axon_hooks")
    mod.get_axon_ntff_profile_hook = lambda: hook
    mod.set_axon_ntff_profile_hook = lambda h: None
    sys.modules["antenv.axon_hooks"] = mod


_install_ntff_shim()


# ---------------- invocation 1: node tables ----------------
def _build_inv1():
    nc = bacc.Bacc("TRN2", target_bir_lowering=False, debug=False,
                   num_devices=NCORES)
    featT = nc.declare_dram_parameter("featT", [P, NODES_PER_CORE], f32,
                                      isOutput=False)
    W_in = nc.declare_dram_parameter("W", [IN, HD], f32, isOutput=False)
    WT_in = nc.declare_dram_parameter("WT", [HD, IN], f32, isOutput=False)
    Al_in = nc.declare_dram_parameter("Al", [HD, 4], f32, isOutput=False)
    Ar_in = nc.declare_dram_parameter("Ar", [HD, 4], f32, isOutput=False)
    # partition-major combined table: col block t holds tile t's [h | el+er]
    hb_out = nc.declare_dram_parameter("hb_out", [P, WIN_PER_CORE * 136], f32,
                                       isOutput=True)

    with tile.TileContext(nc) as tc:
        with tc.tile_pool(name="cst", bufs=1) as cst, \
             tc.tile_pool(name="sb", bufs=4) as sb, \
             tc.tile_pool(name="ps", bufs=6, space="PSUM") as ps, \
             tc.tile_pool(name="psw", bufs=1, space="PSUM") as psw:

            # WLR = [W | Wl | Wr] where Wl = W @ Al, Wr = W @ Ar
            wt_sb = cst.tile([HD, IN], f32, tag="wt")
            nc.sync.dma_start(out=wt_sb[:], in_=WT_in[:])
            al_sb = cst.tile([HD, 4], f32, tag="al")
            nc.sync.dma_start(out=al_sb[:], in_=Al_in[:])
            ar_sb = cst.tile([HD, 4], f32, tag="ar")
            nc.sync.dma_start(out=ar_sb[:], in_=Ar_in[:])

            wlr = cst.tile([IN, 136], f32, tag="wlr")
            nc.sync.dma_start(out=wlr[:, 0:HD], in_=W_in[:])
            wl_ps = psw.tile([IN, 8], f32, tag="wlp")
            nc.tensor.matmul(out=wl_ps[:, 0:4], lhsT=wt_sb[:], rhs=al_sb[:],
                             start=True, stop=True)
            nc.tensor.matmul(out=wl_ps[:, 4:8], lhsT=wt_sb[:], rhs=ar_sb[:],
                             start=True, stop=True)
            nc.scalar.activation(out=wlr[:, 128:136], in_=wl_ps[:],
                                 func=mybir.ActivationFunctionType.Copy)

            CH = 14  # tiles per chunk; 98 = 7 chunks of 14
            n_chunks = NODES_PER_CORE // (P * CH)
            for c in range(n_chunks):
                ft = sb.tile([P, CH * P], f32, tag="ft")
                nc.sync.dma_start(
                    out=ft[:], in_=featT[:, c * CH * P:(c + 1) * CH * P])
                hb = sb.tile([P, CH * 136], f32, tag="hb")
                for t in range(CH):
                    hp = ps.tile([P, 136], f32, tag="hp")
                    nc.tensor.matmul(out=hp[:],
                                     lhsT=ft[:, t * P:(t + 1) * P],
                                     rhs=wlr[:], start=True, stop=True)
                    if t % 2 == 0:
                        nc.scalar.activation(
                            out=hb[:, t * 136:(t + 1) * 136], in_=hp[:],
                            func=mybir.ActivationFunctionType.Copy)
                    else:
                        nc.vector.tensor_copy(
                            hb[:, t * 136:(t + 1) * 136], hp[:])
                nc.sync.dma_start(
                    out=hb_out[:, c * CH * 136:(c + 1) * CH * 136],
                    in_=hb[:])
    nc.compile()
    return nc


# ---------------- invocation 2: edge aggregation ----------------
def _build_inv2(T):
    """T = tiles per window (uniform across all cores/windows)."""
    nc = bacc.Bacc("TRN2", target_bir_lowering=False, debug=False,
                   num_devices=NCORES)
    hsrc = nc.declare_dram_parameter(
        "hsrc", [WIN_PER_CORE * P, T * HD], f32, isOutput=False)
    # partition-major meta: col block w holds window w's [P, T*10]
    meta = nc.declare_dram_parameter(
        "meta", [P, WIN_PER_CORE * T * 10], f32, isOutput=False)
    bias_in = nc.declare_dram_parameter("bias", [1, HD], f32, isOutput=False)
    # partition-major: col block w holds window w's [P, D]; host un-permutes
    out_d = nc.declare_dram_parameter("out", [P, WIN_PER_CORE * D], f32,
                                      isOutput=True)

    KW = T * P  # free width of per-window chunk ops

    with tile.TileContext(nc) as tc:
        with tc.tile_pool(name="cst", bufs=1) as cst, \
             tc.tile_pool(name="ld", bufs=4) as ld, \
             tc.tile_pool(name="wk", bufs=3) as wk, \
             tc.tile_pool(name="bp", bufs=3) as bp, \
             tc.tile_pool(name="fl", bufs=3) as fl, \
             tc.tile_pool(name="gfl", bufs=3) as gfl, \
             tc.tile_pool(name="ob", bufs=1) as ob, \
             tc.tile_pool(name="mt", bufs=1) as mt, \
             tc.tile_pool(name="ps", bufs=3, space="PSUM") as ps, \
             tc.tile_pool(name="psb", bufs=1, space="PSUM") as psb:

            # constants: iota row tile, bias_mean broadcast tile
            iota_row = cst.tile([P, P], f32, tag="iota")
            nc.gpsimd.iota(iota_row[:], pattern=[[1, P]], base=0,
                           channel_multiplier=0,
                           allow_small_or_imprecise_dtypes=True)
            bias_sb = cst.tile([1, HD], f32, tag="brow")
            nc.sync.dma_start(out=bias_sb[:], in_=bias_in[:])
            bias_m = cst.tile([1, D], f32, tag="bm")
            nc.vector.tensor_reduce(
                out=bias_m[:],
                in_=bias_sb[0:1, :].rearrange("p (h d) -> p d h", h=H),
                axis=mybir.AxisListType.X, op=mybir.AluOpType.add)
            nc.vector.tensor_scalar_mul(bias_m[:], bias_m[:], 1.0 / H)
            ones1 = cst.tile([1, P], f32, tag="ones")
            nc.vector.memset(ones1[:], 1.0)
            bias_ps = psb.tile([P, D], f32, tag="bps")
            nc.tensor.matmul(out=bias_ps[:], lhsT=ones1[:], rhs=bias_m[:],
                             start=True, stop=True)
            bias_bc = cst.tile([P, D], f32, tag="bbc")
            nc.vector.tensor_copy(bias_bc[:], bias_ps[:])
            out_sb = ob.tile([P, WIN_PER_CORE * D], f32, tag="osb")
            metaP = mt.tile([P, WIN_PER_CORE * T * 10], f32, tag="mt")
            nc.sync.dma_start(out=metaP[:], in_=meta[:])

            for w in range(WIN_PER_CORE):
                base = w * KW
                # ---- loads (host provides [w*128+p, T*…] contiguous rows)
                hch = ld.tile([P, T * HD], f32, tag="hch")
                nc.sync.dma_start(
                    out=hch[:], in_=hsrc[w * P:(w + 1) * P, :])
                mv = metaP[:, w * T * 10:(w + 1) * T * 10].rearrange(
                    "p (k f) -> p k f", k=T)

                # ---- one-hot B: first TA tiles on ACT, rest on DVE ----
                TA = min(6, T)
                B = bp.tile([P, KW], f32, tag="B")
                for t in range(TA):
                    tmp = fl.tile([P, P], f32, tag="ohtmp")
                    nc.scalar.activation(
                        out=tmp[:], in_=iota_row[:],
                        func=mybir.ActivationFunctionType.Abs,
                        bias=mv[:, t, 9:10])
                    nc.scalar.activation(
                        out=B[:, t * P:(t + 1) * P], in_=tmp[:],
                        func=mybir.ActivationFunctionType.Relu,
                        scale=-1.0, bias=1.0)
                nc.vector.tensor_tensor(
                    out=B[:, TA * P:].rearrange("p (k v) -> p k v", k=T - TA),
                    in0=mv[:, TA:, 8:9].to_broadcast([P, T - TA, P]),
                    in1=iota_row[:].unsqueeze(1).to_broadcast(
                        [P, T - TA, P]),
                    op=mybir.AluOpType.is_equal)

                # ---- logits -> expE, msg ----
                msg = wk.tile([P, T * 132], f32, tag="msg")
                msgv = msg[:].rearrange("p (k f) -> p k f", k=T)
                lg = fl.tile([P, T * 4], f32, tag="lg")
                nc.vector.tensor_tensor(
                    out=lg[:].rearrange("p (k f) -> p k f", k=T),
                    in0=mv[:, :, 0:4], in1=mv[:, :, 4:8],
                    op=mybir.AluOpType.add)
                # exp(leaky(x)) = max(exp(x), exp(NEG*x)) (exp monotone)
                e1 = fl.tile([P, T * 4], f32, tag="e1")
                nc.scalar.activation(out=e1[:], in_=lg[:],
                                     func=mybir.ActivationFunctionType.Exp)
                e2 = fl.tile([P, T * 4], f32, tag="e2")
                nc.scalar.activation(out=e2[:], in_=lg[:], scale=NEG,
                                     func=mybir.ActivationFunctionType.Exp)
                nc.vector.tensor_tensor(
                    out=msgv[:, :, 128:132],
                    in0=e1[:].rearrange("p (k f) -> p k f", k=T),
                    in1=e2[:].rearrange("p (k f) -> p k f", k=T),
                    op=mybir.AluOpType.max)
                # msg[:, :, 0:128] = h * expE (broadcast over D)
                nc.vector.tensor_tensor(
                    out=msgv[:, :, 0:128].rearrange(
                        "p k (h d) -> p k h d", h=H),
                    in0=hch[:].rearrange("p (k h d) -> p k h d", k=T, h=H),
                    in1=msgv[:, :, 128:132].unsqueeze(3).to_broadcast(
                        [P, T, H, D]),
                    op=mybir.AluOpType.mult)

                # ---- accumulate [U | s] over tiles ----
                acc = ps.tile([P, 132], f32, tag="acc")
                for t in range(T):
                    nc.tensor.matmul(
                        out=acc[:],
                        lhsT=B[:, t * P:(t + 1) * P],
                        rhs=msg[:, t * 132:(t + 1) * 132],
                        start=(t == 0), stop=(t == T - 1))

                # ---- flush: out = mean_h(U/s) + bias_mean (ACT-heavy) ----
                r4 = fl.tile([P, 4], f32, tag="r4")
                nc.vector.reciprocal(r4[:], acc[:, 128:132])
                # r4 = min(1/s, 1e30) * (1/H); pad slots (s=0) give inf,
                # clamped here; their rows are discarded by the host anyway
                nc.vector.tensor_scalar(r4[:], r4[:], 1e30, 1.0 / H,
                                        mybir.AluOpType.min,
                                        mybir.AluOpType.mult)
                un = gfl.tile([P, HD], f32, tag="un")
                nc.vector.tensor_tensor(
                    out=un[:].rearrange("p (h dd) -> p h dd", h=H),
                    in0=acc[:, 0:128].rearrange("p (h dd) -> p h dd", h=H),
                    in1=r4[:].unsqueeze(2).to_broadcast([P, H, D]),
                    op=mybir.AluOpType.mult)
                # head-mean + bias on GpSimd (flat tree adds) to unload DVE

                nc.gpsimd.tensor_tensor(
                    out=un[:, 0:64], in0=un[:, 0:64], in1=un[:, 64:128],
                    op=mybir.AluOpType.add)
                nc.gpsimd.tensor_tensor(
                    out=un[:, 0:32], in0=un[:, 0:32], in1=un[:, 32:64],
                    op=mybir.AluOpType.add)
                nc.gpsimd.tensor_tensor(
                    out=out_sb[:, w * D:(w + 1) * D], in0=un[:, 0:32],
                    in1=bias_bc[:], op=mybir.AluOpType.add)

            nc.sync.dma_start(out=out_d[:], in_=out_sb[:])
    nc.compile()
    return nc


_INV1 = None
_INV2 = {}
LAST_EXEC_NS = None
LAST_EXEC_NS1 = None
LAST_EXEC_NS2 = None
import os
_TRACE = bool(os.environ.get("GAT_TRACE"))


def kernel(feat, W, attn_l, attn_r, bias, src, dst):
    global _INV1, LAST_EXEC_NS, LAST_EXEC_NS1, LAST_EXEC_NS2
    feat = np.asarray(feat, dtype=np.float32)
    W = np.asarray(W, dtype=np.float32)
    attn_l = np.asarray(attn_l, dtype=np.float32)
    attn_r = np.asarray(attn_r, dtype=np.float32)
    bias = np.asarray(bias, dtype=np.float32)
    src = np.asarray(src, dtype=np.int32)
    dst = np.asarray(dst, dtype=np.int32)

    # ---------------- host: layout-only prep ----------------
    featT = np.zeros((IN, N_PAD), dtype=np.float32)
    featT[:, :N] = np.ascontiguousarray(feat.T)
    WT = np.ascontiguousarray(W.T)
    Al = np.zeros((HD, H), dtype=np.float32)
    Ar = np.zeros((HD, H), dtype=np.float32)
    for h in range(H):
        Al[h * D:(h + 1) * D, h] = attn_l[h]
        Ar[h * D:(h + 1) * D, h] = attn_r[h]

    # ---------------- inv-1: node tables ----------------
    if _INV1 is None:
        _INV1 = _build_inv1()
    in1 = []
    for c in range(NCORES):
        sl = slice(c * NODES_PER_CORE, (c + 1) * NODES_PER_CORE)
        in1.append({"featT": np.ascontiguousarray(featT[:, sl]),
                    "W": W, "WT": WT, "Al": Al, "Ar": Ar})
    res1 = run_bass_kernel_spmd(_INV1, in1, core_ids=list(range(NCORES)),
                                trace=_TRACE)
    LAST_EXEC_NS1 = res1.exec_time_ns
    hb_all = np.concatenate(
        [r["hb_out"].reshape(P, WIN_PER_CORE, 136).transpose(1, 0, 2)
         .reshape(NODES_PER_CORE, 136) for r in res1.results], axis=0)
    h_full = np.ascontiguousarray(hb_all[:, 0:HD])
    elr_full = np.ascontiguousarray(hb_all[:, HD:HD + 8])

    # ---------------- host: edge slotting (index ops only) ----------------
    # Degree-balanced node->slot assignment: snake-assign nodes (sorted by
    # in-degree desc) across windows so per-window edge counts equalize.
    n_win_tot = NCORES * WIN_PER_CORE
    deg = np.bincount(dst, minlength=N)
    order = np.argsort(-deg, kind="stable")          # nodes, heavy first
    wseq = np.arange(N, dtype=np.int64) % (2 * n_win_tot)
    wseq = np.where(wseq < n_win_tot, wseq, 2 * n_win_tot - 1 - wseq)
    posc = np.zeros(n_win_tot, dtype=np.int64)
    # position of node within its window = running count per window
    posn = np.zeros(N, dtype=np.int64)
    # vectorized running count: for snake pattern, node i is the
    # (i // (2*n_win_tot))*2 + {0,1}-th member of its window... simpler:
    # each full snake pass hits every window exactly twice.
    pass_idx = np.arange(N, dtype=np.int64) // (2 * n_win_tot)
    within = np.arange(N, dtype=np.int64) % (2 * n_win_tot)
    posn = 2 * pass_idx + (within >= n_win_tot)
    node_slot = np.empty(N, dtype=np.int64)
    node_slot[order] = wseq * P + posn
    assert posn.max() < P

    slot_of_dst = node_slot[dst]
    perm = np.argsort(slot_of_dst, kind="stable")
    srcp = src[perm]
    dslot = slot_of_dst[perm]
    win = dslot >> 7
    counts = np.bincount(win, minlength=n_win_tot)
    T = max(1, int(math.ceil(counts.max() / P)))
    win_start = np.zeros(n_win_tot + 1, dtype=np.int64)
    np.cumsum(counts, out=win_start[1:])
    offs = np.arange(E, dtype=np.int64) - win_start[win]
    slot = win * (T * P) + offs

    S_tot = n_win_tot * T * P
    slot_src = np.zeros(S_tot, dtype=np.int64)
    slot_dstg = np.zeros(S_tot, dtype=np.int64)
    slot_dloc = np.full(S_tot, 999.0, dtype=np.float32)
    valid = np.zeros(S_tot, dtype=bool)
    slot_src[slot] = srcp
    slot_dstg[slot] = dst[perm]
    slot_dloc[slot] = (dslot & 127).astype(np.float32)
    valid[slot] = True

    hsrc_all = h_full[slot_src]              # [S_tot, 128]
    hsrc_all[~valid] = 0.0
    el_s = elr_full[slot_src][:, 0:4]
    er_s = elr_full[slot_dstg][:, 4:8]
    el_s[~valid] = 0.0
    er_s[~valid] = 0.0
    meta_all = np.concatenate(
        [el_s, er_s, slot_dloc[:, None], -slot_dloc[:, None]],
        axis=1).astype(np.float32)

    # per-partition-contiguous layouts: row (w*128+p) = concat over t
    hsrc_lay = np.ascontiguousarray(
        hsrc_all.reshape(n_win_tot, T, P, HD).transpose(0, 2, 1, 3)
        .reshape(n_win_tot * P, T * HD))
    meta_lay = np.ascontiguousarray(
        meta_all.reshape(n_win_tot, T, P, 10).transpose(0, 2, 1, 3)
        .reshape(n_win_tot * P, T * 10))

    # ---------------- inv-2: edge aggregation ----------------
    if T not in _INV2:
        _INV2[T] = _build_inv2(T)
    R_core = WIN_PER_CORE * P
    in2 = []
    for c in range(NCORES):
        sl = slice(c * R_core, (c + 1) * R_core)
        in2.append({"hsrc": hsrc_lay[sl],
                    "meta": np.ascontiguousarray(
                        meta_lay[sl].reshape(WIN_PER_CORE, P, T * 10)
                        .transpose(1, 0, 2).reshape(P, -1)),
                    "bias": bias.reshape(1, HD)})
    res2 = run_bass_kernel_spmd(_INV2[T], in2, core_ids=list(range(NCORES)),
                                trace=_TRACE)
    LAST_EXEC_NS2 = res2.exec_time_ns
    if LAST_EXEC_NS1 is not None and LAST_EXEC_NS2 is not None:
        LAST_EXEC_NS = LAST_EXEC_NS1 + LAST_EXEC_NS2
    dev_out = np.concatenate(
        [r["out"].reshape(P, WIN_PER_CORE, D).transpose(1, 0, 2)
         .reshape(NODES_PER_CORE, D) for r in res2.results], axis=0)
    return np.ascontiguousarray(dev_out[node_slot])

